# revision 7
# baseline (speedup 1.0000x reference)
"""Trainium2 Bass kernel for nn_Attention (sparse attention with seq_sizes masking).

Computation (per example b over its own T axis):
    query = lrelu(dec @ Wq.T + bq)                        [B, 128]
    key   = lrelu(LF @ Wk.T + bk)                         [B, T, 128]
    energy= einsum('bk,btk->bt', query, key)              [B, T]
    att   = softmax(energy) * mask;  att /= sum(att)      [B, T]
    value = lrelu(LF @ Wv.T + bv)                         [B, T, 128]
    ctx   = einsum('bt,btc->bc', att, value)              [B, 128]

Strategy: data-parallel over B across 8 cores. Because att[t>=seq]==0 exactly
and the pre-mask softmax normalization cancels, rows t >= seq_sizes[b] never
matter. The host packs only the valid (128-rounded) row-range of each example
into a dense per-core buffer, pre-transposed to [f, t] layout so the f
contraction lands on SBUF partitions. Examples are LPT-balanced across cores
and slot-aligned (slot lengths = cross-core max) so per-example column ranges
are identical on every core -> one SPMD program with static APs.

Device inner loop per 512-column batch keeps the PE stream homogeneous
(all fp32r, N=512, 1 cyc/row): 4 keyT + 4 valueT accumulating matmuls,
1 energy matmul (queryT stationary), 1 ones(8x128) @ scorenum broadcast matmul
that collapses unnormalized scores across the example axis (exact: masked
entries are 0). Activations+bias fuse into ACT Prelu reads from PSUM; exp on
ACT; mask multiply on the otherwise-idle GpSimd; row-sums and the context
product+reduce (valueT * score_bcast, summed over each slot's static column
range) on DVE. Normalization by 1/rowsum folds into the final score scale and
the context PSUM->SBUF copy.
"""

import numpy as np

import concourse.bass as bass
import concourse.mybir as mybir
from concourse import bacc
from concourse.tile import TileContext
from concourse.masks import make_identity
from concourse.bass_utils import run_bass_kernel_spmd

F32 = mybir.dt.float32
F32R = mybir.dt.float32r
AF = mybir.ActivationFunctionType
ALU = mybir.AluOpType

B, T, D_LF, D_DEC, D_KQ, D_CTX = 64, 2048, 512, 256, 128, 128
N_CORES = 8
EPC = B // N_CORES          # examples (slots) per core
P = 128
BATCH = 512
ALPHA = 0.2                 # leaky relu slope


def _build_kernel(n_p: int, slot_starts, slot_ends):
    """SPMD program for packed length n_p; slot_starts/ends are the static
    per-example column ranges (identical across cores)."""
    assert n_p % BATCH == 0
    nb = n_p // BATCH

    nc = bacc.Bacc(None, target_bir_lowering=False)

    LFT = nc.dram_tensor("lft", [nb, P, 4 * BATCH], F32R, kind="ExternalInput")
    WKT = nc.dram_tensor("wkt", [4, P, D_KQ], F32R, kind="ExternalInput")
    WVT = nc.dram_tensor("wvt", [4, P, D_CTX], F32R, kind="ExternalInput")
    WQT = nc.dram_tensor("wqt", [2, P, D_KQ], F32, kind="ExternalInput")
    DECT = nc.dram_tensor("dect", [2, P, EPC], F32, kind="ExternalInput")
    BK = nc.dram_tensor("bk", [P, 1], F32, kind="ExternalInput")
    BV = nc.dram_tensor("bv", [P, 1], F32, kind="ExternalInput")
    BQ = nc.dram_tensor("bq", [P, 1], F32, kind="ExternalInput")
    MASK = nc.dram_tensor("mask", [EPC, n_p], F32, kind="ExternalInput")
    ONES8 = nc.dram_tensor("ones8", [EPC, P], F32R, kind="ExternalInput")

    SCORE = nc.dram_tensor("score", [EPC, n_p], F32R, kind="ExternalOutput")
    CTX = nc.dram_tensor("ctx", [EPC, D_CTX], F32, kind="ExternalOutput")

    # per-batch list of context sub-reduces; per-slot partial columns
    batch_parts = [[] for _ in range(nb)]
    slot_pcols = [[] for _ in range(EPC)]
    pcol = 0
    for e in range(EPC):
        s, t = int(slot_starts[e]), int(slot_ends[e])
        while s < t:
            ib = s // BATCH
            hi = min(t, (ib + 1) * BATCH)
            batch_parts[ib].append((pcol, e, s, hi))
            slot_pcols[e].append(pcol)
            pcol += 1
            s = hi
    n_pcols = pcol

    with TileContext(nc) as tc:
        with (
            tc.tile_pool(name="const", bufs=1) as cpool,
            tc.tile_pool(name="big", bufs=1) as big,
            tc.tile_pool(name="io", bufs=3) as io,
            tc.tile_pool(name="kvp", bufs=3) as kvp,
            tc.tile_pool(name="vp", bufs=4) as vp,
            tc.tile_pool(name="ps", bufs=3, space="PSUM") as ps,
            tc.tile_pool(name="pse", bufs=2, space="PSUM") as pse,
            tc.tile_pool(name="psb", bufs=2, space="PSUM") as psb,
            tc.tile_pool(name="psc", bufs=1, space="PSUM") as psc,
        ):
            # ---- constants ----
            wkt = cpool.tile([P, 4, D_KQ], F32R, tag="wkt")
            nc.sync.dma_start(wkt[:], WKT.ap().rearrange("f p m -> p f m"))
            wvt = cpool.tile([P, 4, D_CTX], F32R, tag="wvt")
            nc.sync.dma_start(wvt[:], WVT.ap().rearrange("f p m -> p f m"))
            wqt = cpool.tile([P, 2, D_KQ], F32, tag="wqt")
            nc.sync.dma_start(wqt[:], WQT.ap().rearrange("f p m -> p f m"))
            dect = cpool.tile([P, 2, EPC], F32, tag="dect")
            nc.sync.dma_start(dect[:], DECT.ap().rearrange("f p m -> p f m"))
            bk = cpool.tile([P, 1], F32, tag="bk")
            nc.sync.dma_start(bk[:], BK[:, :])
            bv = cpool.tile([P, 1], F32, tag="bv")
            nc.sync.dma_start(bv[:], BV[:, :])
            bq = cpool.tile([P, 1], F32, tag="bq")
            nc.sync.dma_start(bq[:], BQ[:, :])
            mask = big.tile([EPC, n_p], F32, tag="mask")
            nc.sync.dma_start(mask[:], MASK[:, :])
            ones8 = cpool.tile([EPC, P], F32R, tag="ones8")
            nc.sync.dma_start(ones8[:], ONES8[:, :])
            ident = cpool.tile([P, P], F32, tag="ident")
            make_identity(nc, ident[:])

            # ---- query: [k, ex] = lrelu(WqT-chunks.T @ decT + bq), fp32r out ----
            psq = psc.tile([P, EPC], F32, tag="ctxq")
            nc.tensor.matmul(psq[:], wqt[:, 0], dect[:, 0], start=True, stop=False)
            nc.tensor.matmul(psq[:], wqt[:, 1], dect[:, 1], start=False, stop=True)
            queryT = cpool.tile([P, EPC], F32R, tag="queryT")
            nc.scalar.activation(queryT[:], psq[:], AF.Prelu,
                                 bias=bq[:], scale=1.0, alpha=ALPHA)

            # ---- residents ----
            escore = big.tile([EPC, n_p], F32R, tag="escore")
            psums = cpool.tile([EPC, nb], F32, tag="psums")
            sums = cpool.tile([EPC, 1], F32, tag="sums")
            recip = cpool.tile([EPC, 1], F32, tag="recip")
            ctx_part = big.tile([P, max(n_pcols, 1)], F32, tag="ctx_part")
            ctx_cols = cpool.tile([P, EPC], F32, tag="ctx_cols")

            # ---- main loop (stage 2 runs with a LAG-batch delay so the
            # PE never stalls waiting for the same batch's exp/mask) ----
            LAG = 2
            valueTs = {}

            def stage2(jb):
                sl2 = slice(jb * BATCH, (jb + 1) * BATCH)
                nc.vector.tensor_reduce(psums[:, jb:jb + 1], escore[:, sl2],
                                        mybir.AxisListType.X, ALU.add)
                # score broadcast across partitions (masked cols are exact 0,
                # so the column-sum over examples recovers the owner's score)
                psbt = psb.tile([P, BATCH], F32, tag="sb")
                nc.tensor.matmul(psbt[:], ones8[:], escore[:, sl2],
                                 start=True, stop=True)
                # context partials: prod = valueT * score_bcast; reduce slots
                prod = vp.tile([P, BATCH], F32, tag="prod")
                nc.vector.tensor_tensor(prod[:], valueTs.pop(jb)[:], psbt[:],
                                        ALU.mult)
                for (pc, e, lo, hi) in batch_parts[jb]:
                    nc.vector.tensor_reduce(
                        ctx_part[:, pc:pc + 1],
                        prod[:, lo - jb * BATCH:hi - jb * BATCH],
                        mybir.AxisListType.X, ALU.add)

            for ib in range(nb):
                sl = slice(ib * BATCH, (ib + 1) * BATCH)
                lft = io.tile([P, 4, BATCH], F32R, tag="lft")
                nc.sync.dma_start(
                    lft[:], LFT.ap()[ib].rearrange("p (f n) -> p f n", f=4))

                psk = ps.tile([P, BATCH], F32, tag="pskv")
                for fc in range(4):
                    nc.tensor.matmul(psk[:], wkt[:, fc], lft[:, fc],
                                     start=(fc == 0), stop=(fc == 3))
                keyT = kvp.tile([P, BATCH], F32R, tag="keyT")
                nc.scalar.activation(keyT[:], psk[:], AF.Prelu,
                                     bias=bk[:], scale=1.0, alpha=ALPHA)

                psv = ps.tile([P, BATCH], F32, tag="pskv")
                for fc in range(4):
                    nc.tensor.matmul(psv[:], wvt[:, fc], lft[:, fc],
                                     start=(fc == 0), stop=(fc == 3))
                valueT = vp.tile([P, BATCH], F32, tag="valueT")
                nc.scalar.activation(valueT[:], psv[:], AF.Prelu,
                                     bias=bv[:], scale=1.0, alpha=ALPHA)
                valueTs[ib] = valueT

                # energy -> exp -> mask (GpSimd)
                pe_ = pse.tile([EPC, BATCH], F32, tag="pe")
                nc.tensor.matmul(pe_[:], queryT[:], keyT[:], start=True, stop=True)
                nc.scalar.activation(escore[:, sl], pe_[:], AF.Exp,
                                     bias=0.0, scale=1.0)
                nc.gpsimd.tensor_tensor(escore[:, sl], escore[:, sl], mask[:, sl],
                                        ALU.mult)

                if ib >= LAG:
                    stage2(ib - LAG)
            for jb in range(max(0, nb - LAG), nb):
                stage2(jb)

            # ---- finalize ----
            nc.vector.tensor_reduce(sums[:], psums[:], mybir.AxisListType.X,
                                    ALU.add)
            nc.vector.reciprocal(recip[:], sums[:])
            half = (nb // 2) * BATCH
            nc.vector.tensor_scalar_mul(escore[:, :half], escore[:, :half],
                                        recip[:])
            nc.sync.dma_start(SCORE[:, :half], escore[:, :half])
            nc.gpsimd.tensor_scalar_mul(escore[:, half:], escore[:, half:],
                                        recip[:])
            nc.sync.dma_start(SCORE[:, half:], escore[:, half:])

            for e in range(EPC):
                pcs = slot_pcols[e]
                if len(pcs) == 1:
                    nc.vector.tensor_copy(ctx_cols[:, e:e + 1],
                                          ctx_part[:, pcs[0]:pcs[0] + 1])
                else:
                    assert pcs == list(range(pcs[0], pcs[-1] + 1))
                    nc.vector.tensor_reduce(
                        ctx_cols[:, e:e + 1],
                        ctx_part[:, pcs[0]:pcs[-1] + 1],
                        mybir.AxisListType.X, ALU.add)

            ctx_ps = psc.tile([EPC, D_CTX], F32, tag="ctxq")
            nc.tensor.transpose(ctx_ps[:], ctx_cols[:], ident[:])
            ctx_sb = cpool.tile([EPC, D_CTX], F32, tag="ctx_sb")
            nc.scalar.activation(ctx_sb[:], ctx_ps[:], AF.Copy,
                                 bias=0.0, scale=recip[:])
            nc.sync.dma_start(CTX[:, :], ctx_sb[:])

    nc.compile()
    return nc


def _pack_inputs(decoder_state, listener_feature, seq_sizes, Wq, bq, Wk, bk, Wv, bv):
    """LPT-balance examples over cores; slot-align (cross-core max slot
    lengths); pre-transpose LF to [f, t] in a batch-local layout."""
    seq = np.asarray(seq_sizes).astype(np.int64)
    tiles = (seq + P - 1) // P

    order = np.argsort(-tiles, kind="stable")
    bins = [[] for _ in range(N_CORES)]
    loads = np.zeros(N_CORES, dtype=np.int64)
    for b_idx in order:
        open_bins = [c for c in range(N_CORES) if len(bins[c]) < EPC]
        c = min(open_bins, key=lambda c: loads[c])
        bins[c].append(int(b_idx))
        loads[c] += tiles[b_idx]
    # slot-align: per core sort desc, slot length = max over cores
    for c in range(N_CORES):
        bins[c].sort(key=lambda b_idx: -tiles[b_idx])
    slot_len = np.zeros(EPC, dtype=np.int64)
    for c in range(N_CORES):
        for e, b_idx in enumerate(bins[c]):
            slot_len[e] = max(slot_len[e], tiles[b_idx])
    slot_rows = slot_len * P
    n_p = int(slot_rows.sum())
    n_p = max(BATCH, ((n_p + BATCH - 1) // BATCH) * BATCH)
    slot_starts = np.concatenate([[0], np.cumsum(slot_rows)])[:EPC]
    slot_ends = slot_starts + slot_rows
    nb = n_p // BATCH

    WkT = np.ascontiguousarray(Wk.T).reshape(4, P, D_KQ)
    WvT = np.ascontiguousarray(Wv.T).reshape(4, P, D_CTX)
    WqT = np.ascontiguousarray(Wq.T).reshape(2, P, D_KQ)
    bk_c = np.ascontiguousarray(bk.reshape(P, 1))
    bv_c = np.ascontiguousarray(bv.reshape(P, 1))
    bq_c = np.ascontiguousarray(bq.reshape(P, 1))
    ones8 = np.ones((EPC, P), dtype=np.float32)

    in_maps, meta = [], []
    for c in range(N_CORES):
        lft = np.zeros((P, 4, n_p), dtype=np.float32)
        msk = np.zeros((EPC, n_p), dtype=np.float32)
        dect = np.zeros((D_DEC, EPC), dtype=np.float32)
        for e, b_idx in enumerate(bins[c]):
            pos = int(slot_starts[e])
            rows = int(tiles[b_idx]) * P
            lf_t = listener_feature[b_idx, :rows, :].T      # [512, rows]
            lft[:, :, pos:pos + rows] = np.transpose(
                lf_t.reshape(4, P, rows), (1, 0, 2))
            msk[e, pos:pos + int(seq[b_idx])] = 1.0
            dect[:, e] = decoder_state[b_idx]
        # batch-local layout: [nb, P, 4*BATCH], per partition contiguous
        lft_b = np.transpose(lft.reshape(P, 4, nb, BATCH), (2, 0, 1, 3))
        in_maps.append({
            "lft": np.ascontiguousarray(lft_b).reshape(nb, P, 4 * BATCH),
            "wkt": WkT, "wvt": WvT, "wqt": WqT,
            "dect": np.ascontiguousarray(dect.reshape(2, P, EPC)),
            "bk": bk_c, "bv": bv_c, "bq": bq_c,
            "mask": msk, "ones8": ones8,
        })
        meta.append(bins[c])
    return in_maps, meta, n_p, slot_starts, slot_ends


def kernel(decoder_state, listener_feature, seq_sizes, Wq, bq, Wk, bk, Wv, bv,
           _trace=False):
    decoder_state = np.asarray(decoder_state, dtype=np.float32)
    listener_feature = np.asarray(listener_feature, dtype=np.float32)
    seq_sizes = np.asarray(seq_sizes)
    Wq = np.asarray(Wq, dtype=np.float32); bq = np.asarray(bq, dtype=np.float32)
    Wk = np.asarray(Wk, dtype=np.float32); bk = np.asarray(bk, dtype=np.float32)
    Wv = np.asarray(Wv, dtype=np.float32); bv = np.asarray(bv, dtype=np.float32)
    in_maps, meta, n_p, slot_starts, slot_ends = _pack_inputs(
        decoder_state, listener_feature, seq_sizes, Wq, bq, Wk, bk, Wv, bv)

    nc = _build_kernel(n_p, slot_starts, slot_ends)
    res = run_bass_kernel_spmd(nc, in_maps, core_ids=list(range(N_CORES)),
                               trace=_trace)

    seq = np.asarray(seq_sizes).astype(np.int64)
    att = np.zeros((B, T), dtype=np.float32)
    ctx = np.zeros((B, D_CTX), dtype=np.float32)
    for c in range(N_CORES):
        score_p = res.results[c]["score"]
        ctx_p = res.results[c]["ctx"]
        for e, b_idx in enumerate(meta[c]):
            s = int(seq[b_idx])
            st = int(slot_starts[e])
            att[b_idx, :s] = score_p[e, st:st + s]
            ctx[b_idx] = ctx_p[e]

    if _trace:
        kernel._last_results = res
    return att, ctx


# revision 8
# speedup vs baseline: 1.8567x; 1.8567x over previous
"""Trainium2 Bass kernel for nn_Attention (sparse attention with seq_sizes masking).

Computation (per example b over its own T axis):
    query = lrelu(dec @ Wq.T + bq)                        [B, 128]
    key   = lrelu(LF @ Wk.T + bk)                         [B, T, 128]
    energy= einsum('bk,btk->bt', query, key)              [B, T]
    att   = softmax(energy) * mask;  att /= sum(att)      [B, T]
    value = lrelu(LF @ Wv.T + bv)                         [B, T, 128]
    ctx   = einsum('bt,btc->bc', att, value)              [B, 128]

Strategy: data-parallel over B across 8 cores. Because att[t>=seq]==0 exactly
and the pre-mask softmax normalization cancels, rows t >= seq_sizes[b] never
matter. The host packs only the valid (128-rounded) row-range of each example
into a dense per-core buffer, pre-transposed to [f, t] layout so the f
contraction lands on SBUF partitions. Examples are LPT-balanced across cores
and slot-aligned (slot lengths = cross-core max) so per-example column ranges
are identical on every core -> one SPMD program with static APs.

Device inner loop per 512-column batch keeps the PE stream homogeneous
(all fp32r, N=512, 1 cyc/row): 4 keyT + 4 valueT accumulating matmuls,
1 energy matmul (queryT stationary), 1 ones(8x128) @ scorenum broadcast matmul
that collapses unnormalized scores across the example axis (exact: masked
entries are 0). Activations+bias fuse into ACT Prelu reads from PSUM; exp on
ACT; mask multiply on the otherwise-idle GpSimd; row-sums and the context
product+reduce (valueT * score_bcast, summed over each slot's static column
range) on DVE. Normalization by 1/rowsum folds into the final score scale and
the context PSUM->SBUF copy.
"""

import numpy as np

import concourse.bass as bass
import concourse.mybir as mybir
from concourse import bacc
from concourse.tile import TileContext
from concourse.masks import make_identity
from concourse.bass_utils import run_bass_kernel_spmd

F32 = mybir.dt.float32
F32R = mybir.dt.float32r
AF = mybir.ActivationFunctionType
ALU = mybir.AluOpType

B, T, D_LF, D_DEC, D_KQ, D_CTX = 64, 2048, 512, 256, 128, 128
N_CORES = 8
EPC = B // N_CORES          # examples (slots) per core
P = 128
BATCH = 512
ALPHA = 0.2                 # leaky relu slope


def _build_kernel(n_p: int, slot_starts, slot_ends):
    """SPMD program for packed length n_p; slot_starts/ends are the static
    per-example column ranges (identical across cores)."""
    assert n_p % BATCH == 0
    nb = n_p // BATCH

    nc = bacc.Bacc(None, target_bir_lowering=False)

    LFT = nc.dram_tensor("lft", [nb, P, 4 * BATCH], F32R, kind="ExternalInput")
    WKT = nc.dram_tensor("wkt", [4, P, D_KQ], F32R, kind="ExternalInput")
    WVT = nc.dram_tensor("wvt", [4, P, D_CTX], F32R, kind="ExternalInput")
    WQT = nc.dram_tensor("wqt", [2, P, D_KQ], F32, kind="ExternalInput")
    DECT = nc.dram_tensor("dect", [2, P, EPC], F32, kind="ExternalInput")
    BK = nc.dram_tensor("bk", [P, 1], F32, kind="ExternalInput")
    BV = nc.dram_tensor("bv", [P, 1], F32, kind="ExternalInput")
    BQ = nc.dram_tensor("bq", [P, 1], F32, kind="ExternalInput")
    MASK = nc.dram_tensor("mask", [EPC, n_p], F32, kind="ExternalInput")
    ONES8 = nc.dram_tensor("ones8", [EPC, P], F32R, kind="ExternalInput")

    SCORE = nc.dram_tensor("score", [EPC, n_p], F32R, kind="ExternalOutput")
    CTX = nc.dram_tensor("ctx", [EPC, D_CTX], F32, kind="ExternalOutput")

    # per-batch list of context sub-reduces; per-slot partial columns
    batch_parts = [[] for _ in range(nb)]
    slot_pcols = [[] for _ in range(EPC)]
    pcol = 0
    for e in range(EPC):
        s, t = int(slot_starts[e]), int(slot_ends[e])
        while s < t:
            ib = s // BATCH
            hi = min(t, (ib + 1) * BATCH)
            batch_parts[ib].append((pcol, e, s, hi))
            slot_pcols[e].append(pcol)
            pcol += 1
            s = hi
    n_pcols = pcol

    with TileContext(nc) as tc:
        with (
            tc.tile_pool(name="const", bufs=1) as cpool,
            tc.tile_pool(name="big", bufs=1) as big,
            tc.tile_pool(name="io", bufs=3) as io,
            tc.tile_pool(name="kvp", bufs=3) as kvp,
            tc.tile_pool(name="vp", bufs=4) as vp,
            tc.tile_pool(name="ps", bufs=3, space="PSUM") as ps,
            tc.tile_pool(name="pse", bufs=2, space="PSUM") as pse,
            tc.tile_pool(name="psb", bufs=2, space="PSUM") as psb,
            tc.tile_pool(name="psc", bufs=1, space="PSUM") as psc,
        ):
            # ---- constants ----
            wkt = cpool.tile([P, 4, D_KQ], F32R, tag="wkt")
            nc.sync.dma_start(wkt[:], WKT.ap().rearrange("f p m -> p f m"))
            wvt = cpool.tile([P, 4, D_CTX], F32R, tag="wvt")
            nc.sync.dma_start(wvt[:], WVT.ap().rearrange("f p m -> p f m"))
            wqt = cpool.tile([P, 2, D_KQ], F32, tag="wqt")
            nc.sync.dma_start(wqt[:], WQT.ap().rearrange("f p m -> p f m"))
            dect = cpool.tile([P, 2, EPC], F32, tag="dect")
            nc.sync.dma_start(dect[:], DECT.ap().rearrange("f p m -> p f m"))
            bk = cpool.tile([P, 1], F32, tag="bk")
            nc.sync.dma_start(bk[:], BK[:, :])
            bv = cpool.tile([P, 1], F32, tag="bv")
            nc.sync.dma_start(bv[:], BV[:, :])
            bq = cpool.tile([P, 1], F32, tag="bq")
            nc.sync.dma_start(bq[:], BQ[:, :])
            mask = big.tile([EPC, n_p], F32, tag="mask")
            nc.sync.dma_start(mask[:], MASK[:, :])
            ones8 = cpool.tile([EPC, P], F32R, tag="ones8")
            nc.sync.dma_start(ones8[:], ONES8[:, :])
            ident = cpool.tile([P, P], F32, tag="ident")
            make_identity(nc, ident[:])

            # ---- query: [k, ex] = lrelu(WqT-chunks.T @ decT + bq), fp32r out ----
            psq = psc.tile([P, EPC], F32, tag="ctxq")
            nc.tensor.matmul(psq[:], wqt[:, 0], dect[:, 0], start=True, stop=False)
            nc.tensor.matmul(psq[:], wqt[:, 1], dect[:, 1], start=False, stop=True)
            queryT = cpool.tile([P, EPC], F32R, tag="queryT")
            nc.scalar.activation(queryT[:], psq[:], AF.Prelu,
                                 bias=bq[:], scale=1.0, alpha=ALPHA)

            # ---- residents ----
            escore = big.tile([EPC, n_p], F32R, tag="escore")
            psums = cpool.tile([EPC, nb], F32, tag="psums")
            sums = cpool.tile([EPC, 1], F32, tag="sums")
            recip = cpool.tile([EPC, 1], F32, tag="recip")
            ctx_part = big.tile([P, max(n_pcols, 1)], F32, tag="ctx_part")
            ctx_cols = cpool.tile([P, EPC], F32, tag="ctx_cols")

            # ---- main loop (stage 2 runs with a LAG-batch delay so the
            # PE never stalls waiting for the same batch's exp/mask) ----
            LAG = 2
            valueTs = {}

            def stage2(jb):
                sl2 = slice(jb * BATCH, (jb + 1) * BATCH)
                nc.vector.tensor_reduce(psums[:, jb:jb + 1], escore[:, sl2],
                                        mybir.AxisListType.X, ALU.add)
                # score broadcast across partitions (masked cols are exact 0,
                # so the column-sum over examples recovers the owner's score)
                psbt = psb.tile([P, BATCH], F32, tag="sb")
                nc.tensor.matmul(psbt[:], ones8[:], escore[:, sl2],
                                 start=True, stop=True)
                # context partials: prod = valueT * score_bcast; reduce slots
                prod = vp.tile([P, BATCH], F32, tag="prod")
                nc.vector.tensor_tensor(prod[:], valueTs.pop(jb)[:], psbt[:],
                                        ALU.mult)
                for (pc, e, lo, hi) in batch_parts[jb]:
                    nc.vector.tensor_reduce(
                        ctx_part[:, pc:pc + 1],
                        prod[:, lo - jb * BATCH:hi - jb * BATCH],
                        mybir.AxisListType.X, ALU.add)

            for ib in range(nb):
                sl = slice(ib * BATCH, (ib + 1) * BATCH)
                lft = io.tile([P, 4, BATCH], F32R, tag="lft")
                nc.sync.dma_start(
                    lft[:], LFT.ap()[ib].rearrange("p (f n) -> p f n", f=4))

                psk = ps.tile([P, BATCH], F32, tag="pskv")
                for fc in range(4):
                    nc.tensor.matmul(psk[:], wkt[:, fc], lft[:, fc],
                                     start=(fc == 0), stop=(fc == 3))
                keyT = kvp.tile([P, BATCH], F32R, tag="keyT")
                nc.scalar.activation(keyT[:], psk[:], AF.Prelu,
                                     bias=bk[:], scale=1.0, alpha=ALPHA)

                psv = ps.tile([P, BATCH], F32, tag="pskv")
                for fc in range(4):
                    nc.tensor.matmul(psv[:], wvt[:, fc], lft[:, fc],
                                     start=(fc == 0), stop=(fc == 3))
                valueT = vp.tile([P, BATCH], F32, tag="valueT")
                nc.scalar.activation(valueT[:], psv[:], AF.Prelu,
                                     bias=bv[:], scale=1.0, alpha=ALPHA)
                valueTs[ib] = valueT

                # energy -> exp -> mask (GpSimd)
                pe_ = pse.tile([EPC, BATCH], F32, tag="pe")
                nc.tensor.matmul(pe_[:], queryT[:], keyT[:], start=True, stop=True)
                nc.scalar.activation(escore[:, sl], pe_[:], AF.Exp,
                                     bias=0.0, scale=1.0)
                nc.gpsimd.tensor_tensor(escore[:, sl], escore[:, sl], mask[:, sl],
                                        ALU.mult)

                if ib >= LAG:
                    stage2(ib - LAG)
            for jb in range(max(0, nb - LAG), nb):
                stage2(jb)

            # ---- finalize ----
            nc.vector.tensor_reduce(sums[:], psums[:], mybir.AxisListType.X,
                                    ALU.add)
            nc.vector.reciprocal(recip[:], sums[:])
            half = (nb // 2) * BATCH
            nc.vector.tensor_scalar_mul(escore[:, :half], escore[:, :half],
                                        recip[:])
            nc.sync.dma_start(SCORE[:, :half], escore[:, :half])
            nc.vector.tensor_scalar_mul(escore[:, half:], escore[:, half:],
                                        recip[:])
            nc.sync.dma_start(SCORE[:, half:], escore[:, half:])

            for e in range(EPC):
                pcs = slot_pcols[e]
                if len(pcs) == 1:
                    nc.vector.tensor_copy(ctx_cols[:, e:e + 1],
                                          ctx_part[:, pcs[0]:pcs[0] + 1])
                else:
                    assert pcs == list(range(pcs[0], pcs[-1] + 1))
                    nc.vector.tensor_reduce(
                        ctx_cols[:, e:e + 1],
                        ctx_part[:, pcs[0]:pcs[-1] + 1],
                        mybir.AxisListType.X, ALU.add)

            ctx_ps = psc.tile([EPC, D_CTX], F32, tag="ctxq")
            nc.tensor.transpose(ctx_ps[:], ctx_cols[:], ident[:])
            ctx_sb = cpool.tile([EPC, D_CTX], F32, tag="ctx_sb")
            nc.scalar.activation(ctx_sb[:], ctx_ps[:], AF.Copy,
                                 bias=0.0, scale=recip[:])
            nc.sync.dma_start(CTX[:, :], ctx_sb[:])

    nc.compile()
    return nc


def _pack_inputs(decoder_state, listener_feature, seq_sizes, Wq, bq, Wk, bk, Wv, bv):
    """LPT-balance examples over cores; slot-align (cross-core max slot
    lengths); pre-transpose LF to [f, t] in a batch-local layout."""
    seq = np.asarray(seq_sizes).astype(np.int64)
    tiles = (seq + P - 1) // P

    order = np.argsort(-tiles, kind="stable")
    bins = [[] for _ in range(N_CORES)]
    loads = np.zeros(N_CORES, dtype=np.int64)
    for b_idx in order:
        open_bins = [c for c in range(N_CORES) if len(bins[c]) < EPC]
        c = min(open_bins, key=lambda c: loads[c])
        bins[c].append(int(b_idx))
        loads[c] += tiles[b_idx]
    # slot-align: per core sort desc, slot length = max over cores
    for c in range(N_CORES):
        bins[c].sort(key=lambda b_idx: -tiles[b_idx])
    slot_len = np.zeros(EPC, dtype=np.int64)
    for c in range(N_CORES):
        for e, b_idx in enumerate(bins[c]):
            slot_len[e] = max(slot_len[e], tiles[b_idx])
    slot_rows = slot_len * P
    n_p = int(slot_rows.sum())
    n_p = max(BATCH, ((n_p + BATCH - 1) // BATCH) * BATCH)
    slot_starts = np.concatenate([[0], np.cumsum(slot_rows)])[:EPC]
    slot_ends = slot_starts + slot_rows
    nb = n_p // BATCH

    WkT = np.ascontiguousarray(Wk.T).reshape(4, P, D_KQ)
    WvT = np.ascontiguousarray(Wv.T).reshape(4, P, D_CTX)
    WqT = np.ascontiguousarray(Wq.T).reshape(2, P, D_KQ)
    bk_c = np.ascontiguousarray(bk.reshape(P, 1))
    bv_c = np.ascontiguousarray(bv.reshape(P, 1))
    bq_c = np.ascontiguousarray(bq.reshape(P, 1))
    ones8 = np.ones((EPC, P), dtype=np.float32)

    in_maps, meta = [], []
    for c in range(N_CORES):
        lft = np.zeros((P, 4, n_p), dtype=np.float32)
        msk = np.zeros((EPC, n_p), dtype=np.float32)
        dect = np.zeros((D_DEC, EPC), dtype=np.float32)
        for e, b_idx in enumerate(bins[c]):
            pos = int(slot_starts[e])
            rows = int(tiles[b_idx]) * P
            lf_t = listener_feature[b_idx, :rows, :].T      # [512, rows]
            lft[:, :, pos:pos + rows] = np.transpose(
                lf_t.reshape(4, P, rows), (1, 0, 2))
            msk[e, pos:pos + int(seq[b_idx])] = 1.0
            dect[:, e] = decoder_state[b_idx]
        # batch-local layout: [nb, P, 4*BATCH], per partition contiguous
        lft_b = np.transpose(lft.reshape(P, 4, nb, BATCH), (2, 0, 1, 3))
        in_maps.append({
            "lft": np.ascontiguousarray(lft_b).reshape(nb, P, 4 * BATCH),
            "wkt": WkT, "wvt": WvT, "wqt": WqT,
            "dect": np.ascontiguousarray(dect.reshape(2, P, EPC)),
            "bk": bk_c, "bv": bv_c, "bq": bq_c,
            "mask": msk, "ones8": ones8,
        })
        meta.append(bins[c])
    return in_maps, meta, n_p, slot_starts, slot_ends


def kernel(decoder_state, listener_feature, seq_sizes, Wq, bq, Wk, bk, Wv, bv,
           _trace=False):
    decoder_state = np.asarray(decoder_state, dtype=np.float32)
    listener_feature = np.asarray(listener_feature, dtype=np.float32)
    seq_sizes = np.asarray(seq_sizes)
    Wq = np.asarray(Wq, dtype=np.float32); bq = np.asarray(bq, dtype=np.float32)
    Wk = np.asarray(Wk, dtype=np.float32); bk = np.asarray(bk, dtype=np.float32)
    Wv = np.asarray(Wv, dtype=np.float32); bv = np.asarray(bv, dtype=np.float32)
    in_maps, meta, n_p, slot_starts, slot_ends = _pack_inputs(
        decoder_state, listener_feature, seq_sizes, Wq, bq, Wk, bk, Wv, bv)

    nc = _build_kernel(n_p, slot_starts, slot_ends)
    res = run_bass_kernel_spmd(nc, in_maps, core_ids=list(range(N_CORES)),
                               trace=_trace)

    seq = np.asarray(seq_sizes).astype(np.int64)
    att = np.zeros((B, T), dtype=np.float32)
    ctx = np.zeros((B, D_CTX), dtype=np.float32)
    for c in range(N_CORES):
        score_p = res.results[c]["score"]
        ctx_p = res.results[c]["ctx"]
        for e, b_idx in enumerate(meta[c]):
            s = int(seq[b_idx])
            st = int(slot_starts[e])
            att[b_idx, :s] = score_p[e, st:st + s]
            ctx[b_idx] = ctx_p[e]

    if _trace:
        kernel._last_results = res
    return att, ctx


# revision 10
# speedup vs baseline: 1.8729x; 1.0087x over previous
"""Trainium2 Bass kernel for nn_Attention (sparse attention with seq_sizes masking).

Computation (per example b over its own T axis):
    query = lrelu(dec @ Wq.T + bq)                        [B, 128]
    key   = lrelu(LF @ Wk.T + bk)                         [B, T, 128]
    energy= einsum('bk,btk->bt', query, key)              [B, T]
    att   = softmax(energy) * mask;  att /= sum(att)      [B, T]
    value = lrelu(LF @ Wv.T + bv)                         [B, T, 128]
    ctx   = einsum('bt,btc->bc', att, value)              [B, 128]

Strategy: data-parallel over B across 8 cores. Because att[t>=seq]==0 exactly
and the pre-mask softmax normalization cancels, rows t >= seq_sizes[b] never
matter. The host packs only the valid (128-rounded) row-range of each example
into a dense per-core buffer, pre-transposed to [f, t] layout so the f
contraction lands on SBUF partitions. Examples are LPT-balanced across cores
and slot-aligned (slot lengths = cross-core max) so per-example column ranges
are identical on every core -> one SPMD program with static APs.

Device inner loop per 512-column batch keeps the PE stream homogeneous
(all fp32r, N=512, 1 cyc/row): 4 keyT + 4 valueT accumulating matmuls,
1 energy matmul (queryT stationary), 1 ones(8x128) @ scorenum broadcast matmul
that collapses unnormalized scores across the example axis (exact: masked
entries are 0). Activations+bias fuse into ACT Prelu reads from PSUM; exp on
ACT; mask multiply on the otherwise-idle GpSimd; row-sums and the context
product+reduce (valueT * score_bcast, summed over each slot's static column
range) on DVE. Normalization by 1/rowsum folds into the final score scale and
the context PSUM->SBUF copy.
"""

import numpy as np

import concourse.bass as bass
import concourse.mybir as mybir
from concourse import bacc
from concourse.tile import TileContext
from concourse.masks import make_identity
from concourse.bass_utils import run_bass_kernel_spmd

F32 = mybir.dt.float32
F32R = mybir.dt.float32r
AF = mybir.ActivationFunctionType
ALU = mybir.AluOpType

B, T, D_LF, D_DEC, D_KQ, D_CTX = 64, 2048, 512, 256, 128, 128
N_CORES = 8
EPC = B // N_CORES          # examples (slots) per core
P = 128
BATCH = 512
ALPHA = 0.2                 # leaky relu slope


def _build_kernel(n_p: int, slot_starts, slot_ends):
    """SPMD program for packed length n_p; slot_starts/ends are the static
    per-example column ranges (identical across cores)."""
    assert n_p % BATCH == 0
    nb = n_p // BATCH

    nc = bacc.Bacc(None, target_bir_lowering=False)

    LFT = nc.dram_tensor("lft", [nb, P, 4 * BATCH], F32R, kind="ExternalInput")
    WKT = nc.dram_tensor("wkt", [4, P, D_KQ], F32R, kind="ExternalInput")
    WVT = nc.dram_tensor("wvt", [4, P, D_CTX], F32R, kind="ExternalInput")
    WQT = nc.dram_tensor("wqt", [2, P, D_KQ], F32, kind="ExternalInput")
    DECT = nc.dram_tensor("dect", [2, P, EPC], F32, kind="ExternalInput")
    BK = nc.dram_tensor("bk", [P, 1], F32, kind="ExternalInput")
    BV = nc.dram_tensor("bv", [P, 1], F32, kind="ExternalInput")
    BQ = nc.dram_tensor("bq", [P, 1], F32, kind="ExternalInput")
    MASK = nc.dram_tensor("mask", [EPC, n_p], F32, kind="ExternalInput")
    ONES8 = nc.dram_tensor("ones8", [EPC, P], F32R, kind="ExternalInput")

    SCORE = nc.dram_tensor("score", [EPC, n_p], F32R, kind="ExternalOutput")
    CTX = nc.dram_tensor("ctx", [EPC, D_CTX], F32, kind="ExternalOutput")

    # per-batch list of context sub-reduces; per-slot partial columns
    batch_parts = [[] for _ in range(nb)]
    slot_pcols = [[] for _ in range(EPC)]
    pcol = 0
    for e in range(EPC):
        s, t = int(slot_starts[e]), int(slot_ends[e])
        while s < t:
            ib = s // BATCH
            hi = min(t, (ib + 1) * BATCH)
            batch_parts[ib].append((pcol, e, s, hi))
            slot_pcols[e].append(pcol)
            pcol += 1
            s = hi
    n_pcols = pcol

    with TileContext(nc) as tc:
        with (
            tc.tile_pool(name="const", bufs=1) as cpool,
            tc.tile_pool(name="big", bufs=1) as big,
            tc.tile_pool(name="io", bufs=3) as io,
            tc.tile_pool(name="kvp", bufs=4) as kvp,
            tc.tile_pool(name="vp", bufs=5) as vp,
            tc.tile_pool(name="ps", bufs=3, space="PSUM") as ps,
            tc.tile_pool(name="pse", bufs=2, space="PSUM") as pse,
            tc.tile_pool(name="psb", bufs=2, space="PSUM") as psb,
            tc.tile_pool(name="psc", bufs=1, space="PSUM") as psc,
        ):
            # ---- constants ----
            wkt = cpool.tile([P, 4, D_KQ], F32R, tag="wkt")
            nc.sync.dma_start(wkt[:], WKT.ap().rearrange("f p m -> p f m"))
            wvt = cpool.tile([P, 4, D_CTX], F32R, tag="wvt")
            nc.sync.dma_start(wvt[:], WVT.ap().rearrange("f p m -> p f m"))
            wqt = cpool.tile([P, 2, D_KQ], F32, tag="wqt")
            nc.sync.dma_start(wqt[:], WQT.ap().rearrange("f p m -> p f m"))
            dect = cpool.tile([P, 2, EPC], F32, tag="dect")
            nc.sync.dma_start(dect[:], DECT.ap().rearrange("f p m -> p f m"))
            bk = cpool.tile([P, 1], F32, tag="bk")
            nc.sync.dma_start(bk[:], BK[:, :])
            bv = cpool.tile([P, 1], F32, tag="bv")
            nc.sync.dma_start(bv[:], BV[:, :])
            bq = cpool.tile([P, 1], F32, tag="bq")
            nc.sync.dma_start(bq[:], BQ[:, :])
            mask = big.tile([EPC, n_p], F32, tag="mask")
            nc.sync.dma_start(mask[:], MASK[:, :])
            ones8 = cpool.tile([EPC, P], F32R, tag="ones8")
            nc.sync.dma_start(ones8[:], ONES8[:, :])
            ident = cpool.tile([P, P], F32, tag="ident")
            make_identity(nc, ident[:])

            # ---- query: [k, ex] = lrelu(WqT-chunks.T @ decT + bq), fp32r out ----
            psq = psc.tile([P, EPC], F32, tag="ctxq")
            nc.tensor.matmul(psq[:], wqt[:, 0], dect[:, 0], start=True, stop=False)
            nc.tensor.matmul(psq[:], wqt[:, 1], dect[:, 1], start=False, stop=True)
            queryT = cpool.tile([P, EPC], F32R, tag="queryT")
            nc.scalar.activation(queryT[:], psq[:], AF.Prelu,
                                 bias=bq[:], scale=1.0, alpha=ALPHA)

            # ---- residents ----
            escore = big.tile([EPC, n_p], F32R, tag="escore")
            psums = cpool.tile([EPC, nb], F32, tag="psums")
            sums = cpool.tile([EPC, 1], F32, tag="sums")
            recip = cpool.tile([EPC, 1], F32, tag="recip")
            ctx_part = big.tile([P, max(n_pcols, 1)], F32, tag="ctx_part")
            ctx_cols = cpool.tile([P, EPC], F32, tag="ctx_cols")

            # ---- main loop, software-pipelined so no PE instruction ever
            # waits on same-batch ACT/DVE work:
            #   stage1(b):  DMA, kv matmuls, Prelus        (PE <- DMA only)
            #   stageE(b):  energy matmul (keyT from b)    (emitted at b+1)
            #   stageS(b):  exp, mask, row-sums            (emitted at b+1)
            #   stage2(b):  score-bcast matmul, prod, ctx  (emitted at b+2)
            keyTs, valueTs = {}, {}

            def stageE(jb):
                pe_ = pse.tile([EPC, BATCH], F32, tag="pe")
                nc.tensor.matmul(pe_[:], queryT[:], keyTs.pop(jb)[:],
                                 start=True, stop=True)
                return pe_

            def stageS(jb, pe_):
                sl1 = slice(jb * BATCH, (jb + 1) * BATCH)
                nc.scalar.activation(escore[:, sl1], pe_[:], AF.Exp,
                                     bias=0.0, scale=1.0)
                nc.gpsimd.tensor_tensor(escore[:, sl1], escore[:, sl1],
                                        mask[:, sl1], ALU.mult)
                nc.vector.tensor_reduce(psums[:, jb:jb + 1], escore[:, sl1],
                                        mybir.AxisListType.X, ALU.add)

            def stage2(jb):
                sl2 = slice(jb * BATCH, (jb + 1) * BATCH)
                # score broadcast across partitions (masked cols are exact 0,
                # so the column-sum over examples recovers the owner's score)
                psbt = psb.tile([P, BATCH], F32, tag="sb")
                nc.tensor.matmul(psbt[:], ones8[:], escore[:, sl2],
                                 start=True, stop=True)
                # context partials: prod = valueT * score_bcast; reduce slots
                prod = vp.tile([P, BATCH], F32, tag="prod")
                nc.vector.tensor_tensor(prod[:], valueTs.pop(jb)[:], psbt[:],
                                        ALU.mult)
                for (pc, e, lo, hi) in batch_parts[jb]:
                    nc.vector.tensor_reduce(
                        ctx_part[:, pc:pc + 1],
                        prod[:, lo - jb * BATCH:hi - jb * BATCH],
                        mybir.AxisListType.X, ALU.add)

            pes = {}
            for ib in range(nb):
                lft = io.tile([P, 4, BATCH], F32R, tag="lft")
                nc.sync.dma_start(
                    lft[:], LFT.ap()[ib].rearrange("p (f n) -> p f n", f=4))

                psk = ps.tile([P, BATCH], F32, tag="pskv")
                for fc in range(4):
                    nc.tensor.matmul(psk[:], wkt[:, fc], lft[:, fc],
                                     start=(fc == 0), stop=(fc == 3))
                if ib >= 1:
                    pes[ib - 1] = stageE(ib - 1)
                psv = ps.tile([P, BATCH], F32, tag="pskv")
                for fc in range(4):
                    nc.tensor.matmul(psv[:], wvt[:, fc], lft[:, fc],
                                     start=(fc == 0), stop=(fc == 3))
                if ib >= 2:
                    stage2(ib - 2)

                keyT = kvp.tile([P, BATCH], F32R, tag="keyT")
                nc.scalar.activation(keyT[:], psk[:], AF.Prelu,
                                     bias=bk[:], scale=1.0, alpha=ALPHA)
                keyTs[ib] = keyT
                valueT = vp.tile([P, BATCH], F32, tag="valueT")
                nc.scalar.activation(valueT[:], psv[:], AF.Prelu,
                                     bias=bv[:], scale=1.0, alpha=ALPHA)
                valueTs[ib] = valueT
                if ib >= 1:
                    stageS(ib - 1, pes.pop(ib - 1))

            pes[nb - 1] = stageE(nb - 1)
            stageS(nb - 1, pes.pop(nb - 1))
            for jb in range(max(0, nb - 2), nb):
                stage2(jb)

            # ---- finalize ----
            nc.vector.tensor_reduce(sums[:], psums[:], mybir.AxisListType.X,
                                    ALU.add)
            nc.vector.reciprocal(recip[:], sums[:])
            qs = [(i * nb // 4) * BATCH for i in range(5)]
            for i in range(4):
                if qs[i] == qs[i + 1]:
                    continue
                qsl = slice(qs[i], qs[i + 1])
                nc.vector.tensor_scalar_mul(escore[:, qsl], escore[:, qsl],
                                            recip[:])
                nc.sync.dma_start(SCORE[:, qsl], escore[:, qsl])

            for e in range(EPC):
                pcs = slot_pcols[e]
                if len(pcs) == 1:
                    nc.vector.tensor_copy(ctx_cols[:, e:e + 1],
                                          ctx_part[:, pcs[0]:pcs[0] + 1])
                else:
                    assert pcs == list(range(pcs[0], pcs[-1] + 1))
                    nc.vector.tensor_reduce(
                        ctx_cols[:, e:e + 1],
                        ctx_part[:, pcs[0]:pcs[-1] + 1],
                        mybir.AxisListType.X, ALU.add)

            ctx_ps = psc.tile([EPC, D_CTX], F32, tag="ctxq")
            nc.tensor.transpose(ctx_ps[:], ctx_cols[:], ident[:])
            ctx_sb = cpool.tile([EPC, D_CTX], F32, tag="ctx_sb")
            nc.scalar.activation(ctx_sb[:], ctx_ps[:], AF.Copy,
                                 bias=0.0, scale=recip[:])
            nc.sync.dma_start(CTX[:, :], ctx_sb[:])

    nc.compile()
    return nc


def _pack_inputs(decoder_state, listener_feature, seq_sizes, Wq, bq, Wk, bk, Wv, bv):
    """LPT-balance examples over cores; slot-align (cross-core max slot
    lengths); pre-transpose LF to [f, t] in a batch-local layout."""
    seq = np.asarray(seq_sizes).astype(np.int64)
    tiles = (seq + P - 1) // P

    order = np.argsort(-tiles, kind="stable")
    bins = [[] for _ in range(N_CORES)]
    loads = np.zeros(N_CORES, dtype=np.int64)
    for b_idx in order:
        open_bins = [c for c in range(N_CORES) if len(bins[c]) < EPC]
        c = min(open_bins, key=lambda c: loads[c])
        bins[c].append(int(b_idx))
        loads[c] += tiles[b_idx]
    # slot-align: per core sort desc, slot length = max over cores
    for c in range(N_CORES):
        bins[c].sort(key=lambda b_idx: -tiles[b_idx])
    slot_len = np.zeros(EPC, dtype=np.int64)
    for c in range(N_CORES):
        for e, b_idx in enumerate(bins[c]):
            slot_len[e] = max(slot_len[e], tiles[b_idx])
    slot_rows = slot_len * P
    n_p = int(slot_rows.sum())
    n_p = max(BATCH, ((n_p + BATCH - 1) // BATCH) * BATCH)
    slot_starts = np.concatenate([[0], np.cumsum(slot_rows)])[:EPC]
    slot_ends = slot_starts + slot_rows
    nb = n_p // BATCH

    WkT = np.ascontiguousarray(Wk.T).reshape(4, P, D_KQ)
    WvT = np.ascontiguousarray(Wv.T).reshape(4, P, D_CTX)
    WqT = np.ascontiguousarray(Wq.T).reshape(2, P, D_KQ)
    bk_c = np.ascontiguousarray(bk.reshape(P, 1))
    bv_c = np.ascontiguousarray(bv.reshape(P, 1))
    bq_c = np.ascontiguousarray(bq.reshape(P, 1))
    ones8 = np.ones((EPC, P), dtype=np.float32)

    in_maps, meta = [], []
    for c in range(N_CORES):
        lft = np.zeros((P, 4, n_p), dtype=np.float32)
        msk = np.zeros((EPC, n_p), dtype=np.float32)
        dect = np.zeros((D_DEC, EPC), dtype=np.float32)
        for e, b_idx in enumerate(bins[c]):
            pos = int(slot_starts[e])
            rows = int(tiles[b_idx]) * P
            lf_t = listener_feature[b_idx, :rows, :].T      # [512, rows]
            lft[:, :, pos:pos + rows] = np.transpose(
                lf_t.reshape(4, P, rows), (1, 0, 2))
            msk[e, pos:pos + int(seq[b_idx])] = 1.0
            dect[:, e] = decoder_state[b_idx]
        # batch-local layout: [nb, P, 4*BATCH], per partition contiguous
        lft_b = np.transpose(lft.reshape(P, 4, nb, BATCH), (2, 0, 1, 3))
        in_maps.append({
            "lft": np.ascontiguousarray(lft_b).reshape(nb, P, 4 * BATCH),
            "wkt": WkT, "wvt": WvT, "wqt": WqT,
            "dect": np.ascontiguousarray(dect.reshape(2, P, EPC)),
            "bk": bk_c, "bv": bv_c, "bq": bq_c,
            "mask": msk, "ones8": ones8,
        })
        meta.append(bins[c])
    return in_maps, meta, n_p, slot_starts, slot_ends


def kernel(decoder_state, listener_feature, seq_sizes, Wq, bq, Wk, bk, Wv, bv,
           _trace=False):
    decoder_state = np.asarray(decoder_state, dtype=np.float32)
    listener_feature = np.asarray(listener_feature, dtype=np.float32)
    seq_sizes = np.asarray(seq_sizes)
    Wq = np.asarray(Wq, dtype=np.float32); bq = np.asarray(bq, dtype=np.float32)
    Wk = np.asarray(Wk, dtype=np.float32); bk = np.asarray(bk, dtype=np.float32)
    Wv = np.asarray(Wv, dtype=np.float32); bv = np.asarray(bv, dtype=np.float32)
    in_maps, meta, n_p, slot_starts, slot_ends = _pack_inputs(
        decoder_state, listener_feature, seq_sizes, Wq, bq, Wk, bk, Wv, bv)

    nc = _build_kernel(n_p, slot_starts, slot_ends)
    res = run_bass_kernel_spmd(nc, in_maps, core_ids=list(range(N_CORES)),
                               trace=_trace)

    seq = np.asarray(seq_sizes).astype(np.int64)
    att = np.zeros((B, T), dtype=np.float32)
    ctx = np.zeros((B, D_CTX), dtype=np.float32)
    for c in range(N_CORES):
        score_p = res.results[c]["score"]
        ctx_p = res.results[c]["ctx"]
        for e, b_idx in enumerate(meta[c]):
            s = int(seq[b_idx])
            st = int(slot_starts[e])
            att[b_idx, :s] = score_p[e, st:st + s]
            ctx[b_idx] = ctx_p[e]

    if _trace:
        kernel._last_results = res
    return att, ctx


# revision 13
# speedup vs baseline: 2.0130x; 1.0748x over previous
"""Trainium2 Bass kernel for nn_Attention (sparse attention with seq_sizes masking).

Computation (per example b over its own T axis):
    query = lrelu(dec @ Wq.T + bq)                        [B, 128]
    key   = lrelu(LF @ Wk.T + bk)                         [B, T, 128]
    energy= einsum('bk,btk->bt', query, key)              [B, T]
    att   = softmax(energy) * mask;  att /= sum(att)      [B, T]
    value = lrelu(LF @ Wv.T + bv)                         [B, T, 128]
    ctx   = einsum('bt,btc->bc', att, value)              [B, 128]

Strategy: data-parallel over B across 8 cores. Because att[t>=seq]==0 exactly
and the pre-mask softmax normalization cancels, rows t >= seq_sizes[b] never
matter. The host packs only the valid (128-rounded) row-range of each example
into a dense per-core buffer, pre-transposed to [f, t] layout so the f
contraction lands on SBUF partitions. Examples are LPT-balanced across cores
and slot-aligned (slot lengths = cross-core max) so per-example column ranges
are identical on every core -> one SPMD program with static APs.

Device inner loop per 512-column batch keeps the PE stream homogeneous
(all fp32r, N=512, 1 cyc/row): 4 keyT + 4 valueT accumulating matmuls,
1 energy matmul (queryT stationary), 1 ones(8x128) @ scorenum broadcast matmul
that collapses unnormalized scores across the example axis (exact: masked
entries are 0). Activations+bias fuse into ACT Prelu reads from PSUM; exp on
ACT; mask multiply on the otherwise-idle GpSimd; row-sums and the context
product+reduce (valueT * score_bcast, summed over each slot's static column
range) on DVE. Normalization by 1/rowsum folds into the final score scale and
the context PSUM->SBUF copy.
"""

import numpy as np

import concourse.bass as bass
import concourse.mybir as mybir
from concourse import bacc
from concourse.tile import TileContext
from concourse.masks import make_identity
from concourse.bass_utils import run_bass_kernel_spmd

F32 = mybir.dt.float32
F32R = mybir.dt.float32r
AF = mybir.ActivationFunctionType
ALU = mybir.AluOpType

B, T, D_LF, D_DEC, D_KQ, D_CTX = 64, 2048, 512, 256, 128, 128
N_CORES = 8
EPC = B // N_CORES          # examples (slots) per core
P = 128
BATCH = 512
ALPHA = 0.2                 # leaky relu slope


def _build_kernel(n_p: int, slot_starts, slot_ends):
    """SPMD program for packed length n_p; slot_starts/ends are the static
    per-example column ranges (identical across cores)."""
    assert n_p % BATCH == 0
    nb = n_p // BATCH

    nc = bacc.Bacc(None, target_bir_lowering=False)

    CW = 4 * D_KQ + 4 * D_CTX + 2 * D_KQ + 2 * EPC + 3
    LFT = nc.dram_tensor("lft", [nb, P, 4 * BATCH], F32R, kind="ExternalInput")
    CONSTS = nc.dram_tensor("consts", [P, CW], F32R, kind="ExternalInput")
    MASK = nc.dram_tensor("mask", [EPC, n_p], mybir.dt.bfloat16,
                          kind="ExternalInput")

    SCORE = nc.dram_tensor("score", [EPC, n_p], F32R, kind="ExternalOutput")
    CTX = nc.dram_tensor("ctx", [EPC, D_CTX], F32, kind="ExternalOutput")

    # per-batch list of context sub-reduces; per-slot partial columns
    batch_parts = [[] for _ in range(nb)]
    slot_pcols = [[] for _ in range(EPC)]
    pcol = 0
    for e in range(EPC):
        s, t = int(slot_starts[e]), int(slot_ends[e])
        while s < t:
            ib = s // BATCH
            hi = min(t, (ib + 1) * BATCH)
            batch_parts[ib].append((pcol, e, s, hi))
            slot_pcols[e].append(pcol)
            pcol += 1
            s = hi
    n_pcols = pcol

    with TileContext(nc) as tc:
        with (
            tc.tile_pool(name="const", bufs=1) as cpool,
            tc.tile_pool(name="big", bufs=1) as big,
            tc.tile_pool(name="io", bufs=3) as io,
            tc.tile_pool(name="kvp", bufs=4) as kvp,
            tc.tile_pool(name="vp", bufs=5) as vp,
            tc.tile_pool(name="ps", bufs=3, space="PSUM") as ps,
            tc.tile_pool(name="pse", bufs=2, space="PSUM") as pse,
            tc.tile_pool(name="psb", bufs=2, space="PSUM") as psb,
            tc.tile_pool(name="psc", bufs=1, space="PSUM") as psc,
        ):
            # ---- constants: one DMA for everything weight-like ----
            consts = cpool.tile([P, CW], F32R, tag="consts")
            nc.sync.dma_start(consts[:], CONSTS[:, :])
            o = 0
            wkt = consts[:, o:o + 4 * D_KQ].rearrange("p (f m) -> p f m", f=4)
            o += 4 * D_KQ
            wvt = consts[:, o:o + 4 * D_CTX].rearrange("p (f m) -> p f m", f=4)
            o += 4 * D_CTX
            wqt = consts[:, o:o + 2 * D_KQ].rearrange("p (f m) -> p f m", f=2)
            wqt = wqt.bitcast(F32)
            o += 2 * D_KQ
            dect = consts[:, o:o + 2 * EPC].rearrange("p (f m) -> p f m", f=2)
            dect = dect.bitcast(F32)
            o += 2 * EPC
            bk = consts[:, o:o + 1].bitcast(F32); o += 1
            bv = consts[:, o:o + 1].bitcast(F32); o += 1
            bq = consts[:, o:o + 1].bitcast(F32); o += 1
            mask = big.tile([EPC, n_p], mybir.dt.bfloat16, tag="mask")
            nc.sync.dma_start(mask[:], MASK[:, :])
            ones8f = cpool.tile([EPC, P], F32, tag="ones8f")
            nc.vector.memset(ones8f[:], 1.0)
            ones8 = cpool.tile([EPC, P], F32R, tag="ones8")
            nc.scalar.copy(ones8[:], ones8f[:])
            ident = cpool.tile([P, P], F32, tag="ident")
            make_identity(nc, ident[:])

            # ---- query: [k, ex] = lrelu(WqT-chunks.T @ decT + bq), fp32r out ----
            psq = psc.tile([P, EPC], F32, tag="ctxq")
            nc.tensor.matmul(psq[:], wqt[:, 0], dect[:, 0], start=True, stop=False)
            nc.tensor.matmul(psq[:], wqt[:, 1], dect[:, 1], start=False, stop=True)
            queryT = cpool.tile([P, EPC], F32R, tag="queryT")
            nc.scalar.activation(queryT[:], psq[:], AF.Prelu,
                                 bias=bq[:], scale=1.0, alpha=ALPHA)

            # ---- residents ----
            escore = big.tile([EPC, n_p], F32R, tag="escore")
            psums = cpool.tile([EPC, nb], F32, tag="psums")
            sums = cpool.tile([EPC, 1], F32, tag="sums")
            recip = cpool.tile([EPC, 1], F32, tag="recip")
            ctx_part = big.tile([P, max(n_pcols, 1)], F32, tag="ctx_part")
            ctx_cols = cpool.tile([P, EPC], F32, tag="ctx_cols")

            # ---- main loop, software-pipelined so no PE instruction ever
            # waits on same-batch ACT/DVE work:
            #   stage1(b):  DMA, kv matmuls, Prelus        (PE <- DMA only)
            #   stageE(b):  energy matmul (keyT from b)    (emitted at b+1)
            #   stageS(b):  exp, mask, row-sums            (emitted at b+1)
            #   stage2(b):  score-bcast matmul, prod, ctx  (emitted at b+2)
            keyTs, valueTs = {}, {}

            def stageE(jb):
                pe_ = pse.tile([EPC, BATCH], F32, tag="pe")
                nc.tensor.matmul(pe_[:], queryT[:], keyTs.pop(jb)[:],
                                 start=True, stop=True)
                return pe_

            def stageS(jb, pe_):
                sl1 = slice(jb * BATCH, (jb + 1) * BATCH)
                nc.scalar.activation(escore[:, sl1], pe_[:], AF.Exp,
                                     bias=0.0, scale=1.0)
                nc.gpsimd.tensor_tensor(escore[:, sl1], escore[:, sl1],
                                        mask[:, sl1], ALU.mult)
                nc.vector.tensor_reduce(psums[:, jb:jb + 1], escore[:, sl1],
                                        mybir.AxisListType.X, ALU.add)

            def stage2(jb):
                sl2 = slice(jb * BATCH, (jb + 1) * BATCH)
                # score broadcast across partitions (masked cols are exact 0,
                # so the column-sum over examples recovers the owner's score)
                psbt = psb.tile([P, BATCH], F32, tag="sb")
                nc.tensor.matmul(psbt[:], ones8[:], escore[:, sl2],
                                 start=True, stop=True)
                # context partials: prod = valueT * score_bcast; reduce slots
                prod = vp.tile([P, BATCH], F32, tag="prod")
                nc.vector.tensor_tensor(prod[:], valueTs.pop(jb)[:], psbt[:],
                                        ALU.mult)
                for (pc, e, lo, hi) in batch_parts[jb]:
                    nc.vector.tensor_reduce(
                        ctx_part[:, pc:pc + 1],
                        prod[:, lo - jb * BATCH:hi - jb * BATCH],
                        mybir.AxisListType.X, ALU.add)

            pes = {}
            for ib in range(nb):
                lft = io.tile([P, 4, BATCH], F32R, tag="lft")
                nc.sync.dma_start(
                    lft[:], LFT.ap()[ib].rearrange("p (f n) -> p f n", f=4))

                psk = ps.tile([P, BATCH], F32, tag="pskv")
                for fc in range(4):
                    nc.tensor.matmul(psk[:], wkt[:, fc], lft[:, fc],
                                     start=(fc == 0), stop=(fc == 3))
                if ib >= 1:
                    pes[ib - 1] = stageE(ib - 1)
                psv = ps.tile([P, BATCH], F32, tag="pskv")
                for fc in range(4):
                    nc.tensor.matmul(psv[:], wvt[:, fc], lft[:, fc],
                                     start=(fc == 0), stop=(fc == 3))
                if ib >= 2:
                    stage2(ib - 2)

                keyT = kvp.tile([P, BATCH], F32R, tag="keyT")
                nc.scalar.activation(keyT[:], psk[:], AF.Prelu,
                                     bias=bk[:], scale=1.0, alpha=ALPHA)
                keyTs[ib] = keyT
                valueT = vp.tile([P, BATCH], F32, tag="valueT")
                nc.scalar.activation(valueT[:], psv[:], AF.Prelu,
                                     bias=bv[:], scale=1.0, alpha=ALPHA)
                valueTs[ib] = valueT
                if ib >= 1:
                    stageS(ib - 1, pes.pop(ib - 1))

            pes[nb - 1] = stageE(nb - 1)
            stageS(nb - 1, pes.pop(nb - 1))
            for jb in range(max(0, nb - 2), nb):
                stage2(jb)

            # ---- finalize ----
            nc.vector.tensor_reduce(sums[:], psums[:], mybir.AxisListType.X,
                                    ALU.add)
            nc.vector.reciprocal(recip[:], sums[:])

            for e in range(EPC):
                pcs = slot_pcols[e]
                if len(pcs) == 1:
                    nc.vector.tensor_copy(ctx_cols[:, e:e + 1],
                                          ctx_part[:, pcs[0]:pcs[0] + 1])
                else:
                    assert pcs == list(range(pcs[0], pcs[-1] + 1))
                    nc.vector.tensor_reduce(
                        ctx_cols[:, e:e + 1],
                        ctx_part[:, pcs[0]:pcs[-1] + 1],
                        mybir.AxisListType.X, ALU.add)

            ctx_ps = psc.tile([EPC, D_CTX], F32, tag="ctxq")
            nc.tensor.transpose(ctx_ps[:], ctx_cols[:], ident[:])
            ctx_sb = cpool.tile([EPC, D_CTX], F32, tag="ctx_sb")
            nc.scalar.activation(ctx_sb[:], ctx_ps[:], AF.Copy,
                                 bias=0.0, scale=recip[:])
            nc.sync.dma_start(CTX[:, :], ctx_sb[:])

            qs = [(i * nb // 4) * BATCH for i in range(5)]
            for i in range(4):
                if qs[i] == qs[i + 1]:
                    continue
                qsl = slice(qs[i], qs[i + 1])
                nc.vector.tensor_scalar_mul(escore[:, qsl], escore[:, qsl],
                                            recip[:])
                nc.sync.dma_start(SCORE[:, qsl], escore[:, qsl])

    nc.compile()
    return nc


def _pack_inputs(decoder_state, listener_feature, seq_sizes, Wq, bq, Wk, bk, Wv, bv):
    """LPT-balance examples over cores; slot-align (cross-core max slot
    lengths); pre-transpose LF to [f, t] in a batch-local layout."""
    seq = np.asarray(seq_sizes).astype(np.int64)
    tiles = (seq + P - 1) // P

    order = np.argsort(-tiles, kind="stable")
    bins = [[] for _ in range(N_CORES)]
    loads = np.zeros(N_CORES, dtype=np.int64)
    for b_idx in order:
        open_bins = [c for c in range(N_CORES) if len(bins[c]) < EPC]
        c = min(open_bins, key=lambda c: loads[c])
        bins[c].append(int(b_idx))
        loads[c] += tiles[b_idx]
    # slot-align: per core sort desc, slot length = max over cores
    for c in range(N_CORES):
        bins[c].sort(key=lambda b_idx: -tiles[b_idx])
    slot_len = np.zeros(EPC, dtype=np.int64)
    for c in range(N_CORES):
        for e, b_idx in enumerate(bins[c]):
            slot_len[e] = max(slot_len[e], tiles[b_idx])
    slot_rows = slot_len * P
    n_p = int(slot_rows.sum())
    n_p = max(BATCH, ((n_p + BATCH - 1) // BATCH) * BATCH)
    slot_starts = np.concatenate([[0], np.cumsum(slot_rows)])[:EPC]
    slot_ends = slot_starts + slot_rows
    nb = n_p // BATCH

    import ml_dtypes
    WkT = np.transpose(Wk.T.reshape(4, P, D_KQ), (1, 0, 2)).reshape(P, 4 * D_KQ)
    WvT = np.transpose(Wv.T.reshape(4, P, D_CTX), (1, 0, 2)).reshape(P, 4 * D_CTX)
    WqT = np.transpose(Wq.T.reshape(2, P, D_KQ), (1, 0, 2)).reshape(P, 2 * D_KQ)

    in_maps, meta = [], []
    for c in range(N_CORES):
        lft = np.zeros((P, 4, n_p), dtype=np.float32)
        msk = np.zeros((EPC, n_p), dtype=np.float32)
        dect = np.zeros((2, P, EPC), dtype=np.float32)
        for e, b_idx in enumerate(bins[c]):
            pos = int(slot_starts[e])
            rows = int(tiles[b_idx]) * P
            lf_t = listener_feature[b_idx, :rows, :].T      # [512, rows]
            lft[:, :, pos:pos + rows] = np.transpose(
                lf_t.reshape(4, P, rows), (1, 0, 2))
            msk[e, pos:pos + int(seq[b_idx])] = 1.0
            dect[:, :, e] = decoder_state[b_idx].reshape(2, P)
        # batch-local layout: [nb, P, 4*BATCH], per partition contiguous
        lft_b = np.transpose(lft.reshape(P, 4, nb, BATCH), (2, 0, 1, 3))
        consts = np.concatenate([
            WkT, WvT, WqT,
            np.transpose(dect, (1, 0, 2)).reshape(P, 2 * EPC),
            bk.reshape(P, 1), bv.reshape(P, 1), bq.reshape(P, 1),
        ], axis=1).astype(np.float32)
        in_maps.append({
            "lft": np.ascontiguousarray(lft_b).reshape(nb, P, 4 * BATCH),
            "consts": np.ascontiguousarray(consts),
            "mask": msk.astype(ml_dtypes.bfloat16),
        })
        meta.append(bins[c])
    return in_maps, meta, n_p, slot_starts, slot_ends


def kernel(decoder_state, listener_feature, seq_sizes, Wq, bq, Wk, bk, Wv, bv,
           _trace=False):
    decoder_state = np.asarray(decoder_state, dtype=np.float32)
    listener_feature = np.asarray(listener_feature, dtype=np.float32)
    seq_sizes = np.asarray(seq_sizes)
    Wq = np.asarray(Wq, dtype=np.float32); bq = np.asarray(bq, dtype=np.float32)
    Wk = np.asarray(Wk, dtype=np.float32); bk = np.asarray(bk, dtype=np.float32)
    Wv = np.asarray(Wv, dtype=np.float32); bv = np.asarray(bv, dtype=np.float32)
    in_maps, meta, n_p, slot_starts, slot_ends = _pack_inputs(
        decoder_state, listener_feature, seq_sizes, Wq, bq, Wk, bk, Wv, bv)

    nc = _build_kernel(n_p, slot_starts, slot_ends)
    res = run_bass_kernel_spmd(nc, in_maps, core_ids=list(range(N_CORES)),
                               trace=_trace)

    seq = np.asarray(seq_sizes).astype(np.int64)
    att = np.zeros((B, T), dtype=np.float32)
    ctx = np.zeros((B, D_CTX), dtype=np.float32)
    for c in range(N_CORES):
        score_p = res.results[c]["score"]
        ctx_p = res.results[c]["ctx"]
        for e, b_idx in enumerate(meta[c]):
            s = int(seq[b_idx])
            st = int(slot_starts[e])
            att[b_idx, :s] = score_p[e, st:st + s]
            ctx[b_idx] = ctx_p[e]

    if _trace:
        kernel._last_results = res
    return att, ctx
